# revision 1
# baseline (speedup 1.0000x reference)
"""Trainium2 Bass kernel for a pre-LN transformer decode layer.

nn_DecodeLayer: x [4, 2048, 1024] f32, 16 heads, causal attention, 4x MLP.

Sharding: 8 cores = 4 batch x 2 query-shards. Core c handles batch c%4 and
query tiles {2j + c//4 : j in 0..7} (interleaved 128-row tiles, which
balances causal attention work across the two shards of a batch while
keeping a single uniform SPMD program; all per-core differences are data).

On-chip layout: activations are kept transposed ([e, seq]) so every GEMM
contracts over the partition dim. Attention scores are computed directly in
[key, query] layout (scoresT = K @ Q^T), so softmax'd probabilities feed the
attn@V matmul with no on-chip transposes; softmax denominators come free via
a ones-column appended to V; the causal mask only ever touches the diagonal
128x128 block of each key-tile's query-suffix and is passed in as a tiny
[128, 2, 128] per-core tensor. LN gains/biases and the 1/sqrt(d) scale are
folded into the weights/biases on the host (mathematically exact).
"""

import sys

for _p in ("/opt/trn_rl_repo",):
    if _p not in sys.path:
        sys.path.insert(0, _p)

import numpy as np
import ml_dtypes

import concourse.bass as bass
import concourse.tile as tile
from concourse import bacc, mybir
from concourse.bass_utils import run_bass_kernel_spmd

F32 = mybir.dt.float32
F32R = mybir.dt.float32r
BF16 = mybir.dt.bfloat16

E = 1024          # d_model
S = 2048          # sequence length
BATCH = 4
NH = 16           # heads
HD = 64           # head dim
P = 128
ET = E // P       # 8 e-tiles
QC = 1024         # queries per core
NKT = S // P      # 16 key tiles
FF = 4 * E        # 4096
HT = FF // P      # 32 hidden tiles
N_CORES = 8
EPS = 1e-5
NEG = -1.0e9      # additive mask value


def _r(ap):
    """fp32 -> fp32r view for full-rate TensorE matmuls."""
    return ap.bitcast(F32R)


def _jmin(b):
    # first query-slot whose 128-row tile can see key-tile b, uniform over
    # both parities (the extra slot a parity-0 core computes for odd b is
    # fully masked by the mask input).
    return b // 2


def _segs(q0):
    """Split [q0, 1024) at the PSUM bank boundary (512 f32 cols)."""
    segs = []
    if q0 < 512:
        segs.append((q0, 512 - q0))
    segs.append((max(512, q0), QC - max(512, q0)))
    return segs


def build_program(repeat=1, gelu_mode="hw", phases=None):
    nc = bacc.Bacc("TRN2", num_devices=N_CORES)

    d = {}
    def din(name, shape, dtype):
        d[name] = nc.dram_tensor(name, shape, dtype, kind="ExternalInput").ap()

    din("x_full_bf", [E, S], BF16)     # x[b].T, bf16 (layernorm input)
    din("x_chunk", [E, QC], F32)       # this core's query rows (f32 residual)
    din("x_chunk_bf", [E, QC], BF16)
    din("wq", [E, E], BF16)            # ln1_g-folded, /8-folded
    din("wk", [E, E], BF16)
    din("wv", [E, E], BF16)
    din("wproj", [E, E], BF16)
    din("wfc", [E, FF], BF16)          # ln2_g-folded
    din("wfc2", [FF, E], BF16)
    din("bq", [E], F32)
    din("bk", [E], F32)
    din("bv", [E], F32)
    din("bproj", [E], F32)
    din("bfc", [FF], F32)
    din("bfc2", [E], F32)
    din("mask", [P, 2 * P], BF16)      # multiplicative 0/1 mask
    out_ap = nc.dram_tensor("out", [E, QC], F32, kind="ExternalOutput").ap()

    with tile.TileContext(nc) as tc:
        if repeat == 1:
            _emit(nc, tc, d, out_ap, gelu_mode, phases)
        else:
            with tc.For_i(0, repeat, 1):
                _emit(nc, tc, d, out_ap, gelu_mode, phases)

    nc.compile()
    return nc


def _emit(nc, tc, d, out_ap, gelu_mode="hw", phases=None):
    if phases is None:
        phases = {"ln1", "kqv", "attn", "proj", "ffn"}
    import contextlib
    A = mybir.ActivationFunctionType
    O = mybir.AluOpType
    ctx = contextlib.ExitStack()
    with ctx:
        # --- long-lived pools ---
        pconst = ctx.enter_context(tc.tile_pool(name="pconst", bufs=1))
        pbig = ctx.enter_context(tc.tile_pool(name="pbig", bufs=1))
        pxb = ctx.enter_context(tc.tile_pool(name="pxb", bufs=3))
        prows = ctx.enter_context(tc.tile_pool(name="prows", bufs=2))
        postg = ctx.enter_context(tc.tile_pool(name="postg", bufs=3))

        # --- constants ---
        ones_mat = pconst.tile([P, P], BF16, tag="ones")
        nc.vector.memset(ones_mat, 1.0)
        eps_t = pconst.tile([P, 1], F32, tag="eps")
        nc.vector.memset(eps_t, EPS)
        mask_sb = pconst.tile([P, 2 * P], BF16, tag="mask")
        nc.sync.dma_start(out=mask_sb, in_=d["mask"])

        def bias_cols(name, n_tiles):
            t = pconst.tile([P, n_tiles], F32, tag=f"b_{name}", name=f"b_{name}")
            nc.sync.dma_start(out=t, in_=d[name].rearrange("(t p) -> p t", p=P))
            return t

        bq_sb = bias_cols("bq", ET)
        bk_sb = bias_cols("bk", ET)
        bproj_sb = bias_cols("bproj", ET)
        bfc2_sb = bias_cols("bfc2", ET)
        bfc_sb = bias_cols("bfc", HT)

        # --- layernorm: wide stats via all-ones stationary; everything
        # 128-partition-wide, no partition broadcasts ---
        def layernorm(xn_targets, scols, pst, pbc, pxh, src_dram=None,
                      src_tiles=None, step=512):
            for h0 in range(0, scols, step):
                hw = min(step, scols - h0)
                if src_dram is not None:
                    xhalf = pxh.tile([P, ET, 512], BF16, tag="xh",
                                     name="xh")[:, :, :hw]
                    nc.sync.dma_start(
                        out=xhalf,
                        in_=src_dram[:, h0:h0 + hw]
                        .rearrange("(t p) c -> p t c", p=P))
                ps_x = pst.tile([P, 512], F32, tag="st_x", name="st_x")[:, :hw]
                ps_q = pst.tile([P, 512], F32, tag="st_q", name="st_q")[:, :hw]
                for et in range(ET):
                    if src_dram is not None:
                        xb = xhalf[:, et, :]
                    else:
                        xt = src_tiles(et, h0, hw)
                        xb = pxb.tile([P, 512], BF16, tag="xb",
                                      name="xb")[:, :hw]
                        nc.gpsimd.tensor_copy(xb, xt)
                    sq = pxb.tile([P, 512], BF16, tag="xb", name="xb")[:, :hw]
                    nc.scalar.activation(sq, xb, A.Square)
                    for c0 in range(0, hw, 512):
                        sl = slice(c0, c0 + 512)
                        nc.tensor.matmul(ps_x[:, sl], ones_mat, xb[:, sl],
                                         start=(et == 0), stop=(et == ET - 1))
                        nc.tensor.matmul(ps_q[:, sl], ones_mat, sq[:, sl],
                                         start=(et == 0), stop=(et == ET - 1))
                m_t = pbc.tile([P, 512], BF16, tag="bc", name="bc")[:, :hw]
                nc.scalar.activation(m_t, ps_x, A.Copy, scale=1.0 / E)
                e2_t = pbc.tile([P, 512], F32, tag="bcf", name="bcf")[:, :hw]
                nc.scalar.activation(e2_t, ps_q, A.Copy, scale=1.0 / E)
                var_t = pbc.tile([P, 512], F32, tag="bcf", name="bcf")[:, :hw]
                nc.vector.scalar_tensor_tensor(var_t, in0=m_t, scalar=-1.0,
                                               in1=m_t, op0=O.mult, op1=O.mult)
                nc.vector.tensor_add(var_t, var_t, e2_t)
                nc.scalar.activation(var_t, var_t, A.Sqrt, bias=eps_t)
                r_t = pbc.tile([P, 512], F32, tag="bcf", name="bcf")[:, :hw]
                nc.vector.reciprocal(r_t, var_t)
                for (xn, lo, hi) in xn_targets:
                    a, b = max(lo, h0), min(hi, h0 + hw)
                    if a >= b:
                        continue
                    for et in range(ET):
                        if src_dram is not None:
                            xt = xhalf[:, et, a - h0:b - h0]
                        else:
                            xt = src_tiles(et, a, b - a)
                        dst = xn[:, et, a - lo:b - lo]
                        eng = nc.gpsimd if et % 4 == 3 else nc.vector
                        eng.tensor_sub(dst, xt, m_t[:, a - h0:b - h0])
                        eng.tensor_mul(dst, dst, r_t[:, a - h0:b - h0])

        # ---- phase 1: LN1 on x (bf16 copies of x come from the host) ----
        xnf_h = [pbig.tile([P, ET, 1024], BF16, tag=f"T1{i}", name=f"T1{i}")
                 for i in range(2)]                    # slots reused by x2 halves
        xnc = pbig.tile([P, ET, QC], BF16, tag="T2")   # -> attnT
        if "ln1" in phases:
            with tc.tile_pool(name="pst1", bufs=1, space="PSUM") as pst, \
                 tc.tile_pool(name="pbc1", bufs=4) as pbc, \
                 tc.tile_pool(name="pxh1", bufs=2) as pxh:
                layernorm([(xnf_h[0], 0, 1024), (xnf_h[1], 1024, 2048)], S,
                          pst, pbc, pxh, src_dram=d["x_full_bf"])
                layernorm([(xnc, 0, 1024)], QC, pst, pbc, pxh,
                          src_dram=d["x_chunk_bf"])

        KT = pbig.tile([P, ET, S], BF16, tag="T3")        # -> xn2
        QT = pbig.tile([P, ET, QC], BF16, tag="T4")
        VA = pbig.tile([P, NKT, NH * (HD + 1)], BF16, tag="T5")   # -> Hsb
        if "kqv" in phases:
          with tc.tile_pool(name="pwc", bufs=3) as pw, \
             tc.tile_pool(name="pprobs", bufs=4) as pprobs, \
             tc.tile_pool(name="prb", bufs=2) as prb, \
             tc.tile_pool(name="ppkqv", bufs=2, space="PSUM") as ppk:
            # --- V first (everything downstream of it), natural layout + ones col ---
            bvrow = prows.tile([1, 1024], F32, tag="rows", name="rows")
            nc.sync.dma_start(out=bvrow, in_=d["bv"].rearrange("(o n) -> o n", o=1))
            bvrow_bf = prows.tile([1, 1024], BF16, tag="rows_bf", name="rows_bf")
            nc.gpsimd.tensor_copy(bvrow_bf, bvrow)
            bvb = pw.tile([P, E], BF16, tag="bvb", bufs=1)
            nc.gpsimd.partition_broadcast(bvb, bvrow_bf)
            for vh in range(2):
                hbase = vh * (NH // 2)
                wv_sb = pw.tile([P, ET, 512], BF16, tag="wv", bufs=1, name="wv")
                nc.sync.dma_start(
                    out=wv_sb,
                    in_=d["wv"][:, vh * 512:(vh + 1) * 512]
                    .rearrange("(t p) c -> p t c", p=P))
                for t in range(NKT):
                    ps = ppk.tile([P, 512], F32, tag="mm", name="mm")
                    xn_src = xnf_h[t // 8]
                    for et in range(ET):
                        nc.tensor.matmul(
                            ps, xn_src[:, et, (t % 8) * P:(t % 8 + 1) * P],
                            wv_sb[:, et, :],
                            start=(et == 0), stop=(et == ET - 1))
                    va_v = VA[:, t, hbase * (HD + 1):(hbase + 8) * (HD + 1)] \
                        .rearrange("p (h c) -> p h c", c=HD + 1)
                    nc.vector.tensor_add(
                        va_v[:, :, 0:HD],
                        ps.rearrange("p (h c) -> p h c", c=HD),
                        bvb[:, vh * 512:(vh + 1) * 512]
                        .rearrange("p (h c) -> p h c", c=HD))
                    nc.gpsimd.memset(va_v[:, :, HD:HD + 1], 1.0)

            # --- per kd-tile: K, Q, then the two heads that consume them, so
            # ACT exp of round i overlaps PE matmuls of round i+1 ---
            attnT = pbig.tile([P, ET, QC], BF16, tag="T2")
            for kd in range(ET):
                for (wname, bcol, dst, src_t, scols) in (
                        ("wk", bk_sb, KT, None, S), ("wq", bq_sb, QT, xnc, QC)):
                    wt = pw.tile([P, ET, P], BF16, tag="wcol", name="wcol")
                    nc.sync.dma_start(
                        out=wt,
                        in_=d[wname][:, kd * P:(kd + 1) * P]
                        .rearrange("(t p) c -> p t c", p=P))
                    for c0 in range(0, scols, 512):
                        ps = ppk.tile([P, 512], F32, tag="mm", name="mm")
                        for et in range(ET):
                            if src_t is None:
                                s_ap = xnf_h[c0 // 1024][:, et,
                                                         c0 % 1024:c0 % 1024 + 512]
                            else:
                                s_ap = src_t[:, et, c0:c0 + 512]
                            nc.tensor.matmul(ps, wt[:, et, :], s_ap,
                                             start=(et == 0), stop=(et == ET - 1))
                        nc.vector.tensor_scalar(
                            dst[:, kd, c0:c0 + 512], ps,
                            bcol[:, kd:kd + 1], None, op0=O.add)
                if "attn" not in phases:
                    continue
                for h in (2 * kd, 2 * kd + 1):
                    kdt, off = h // 2, (h % 2) * HD
                    psO = ppk.tile([HD + 1, QC], F32, tag="psO", bufs=1,
                                   name="psO")
                    for b in range(NKT):
                        q0 = _jmin(b) * P
                        qlen = QC - q0
                        probs = pprobs.tile([P, QC], BF16, tag="probs",
                                            name="probs")
                        base = (q0 // 512) * 512
                        ps = ppk.tile([P, 1024], F32, tag="sc", bufs=2,
                                      name="sc")
                        for (s0, ln) in _segs(q0):
                            nc.tensor.matmul(
                                ps[:, s0 - base:s0 - base + ln],
                                KT[off:off + HD, kdt, b * P:(b + 1) * P],
                                QT[off:off + HD, kdt, s0:s0 + ln],
                                start=True, stop=True)
                        nc.scalar.activation(probs[:, 0:qlen],
                                             ps[:, q0 - base:q0 - base + qlen],
                                             A.Exp)
                        mvar = mask_sb[:, (b % 2) * P:(b % 2 + 1) * P]
                        nc.vector.tensor_mul(probs[:, 0:P], probs[:, 0:P],
                                             mvar)
                        for (s0, ln) in _segs(q0):
                            last = 7 if s0 < 512 else 15
                            nc.tensor.matmul(
                                psO[:, s0:s0 + ln],
                                VA[:, b, h * (HD + 1):(h + 1) * (HD + 1)],
                                probs[:, s0 - q0:s0 - q0 + ln],
                                start=(b == 0), stop=(b == last),
                                skip_group_check=True)
                    srow = prows.tile([1, 1024], F32, tag="rows", name="rows")
                    nc.vector.reciprocal(srow, psO[HD:HD + 1, :])
                    srow_bf = prows.tile([1, 1024], BF16, tag="rows_bf",
                                         name="rows_bf")
                    nc.gpsimd.tensor_copy(srow_bf, srow)
                    rb = prb.tile([HD, QC], BF16, tag="rb", name="rb")
                    nc.gpsimd.partition_broadcast(rb, srow_bf)
                    nc.vector.tensor_mul(attnT[off:off + HD, kdt, :],
                                         psO[0:HD, :], rb)

        # ---- phase 4: proj + residual -> x2 (halves), then LN2 -> xn2 ----
        x2_h = [pbig.tile([P, ET, 512], F32, tag=f"T1{i}", name=f"x2{i}")
                for i in range(2)]
        if "proj" in phases:
          with tc.tile_pool(name="pwp", bufs=3) as pw, \
             tc.tile_pool(name="pppr", bufs=1, space="PSUM") as ppp:
            for et in range(ET):
                wt = pw.tile([P, ET, P], BF16, tag="wcol", name="wcol")
                nc.sync.dma_start(
                    out=wt,
                    in_=d["wproj"][:, et * P:(et + 1) * P]
                    .rearrange("(t p) c -> p t c", p=P))
                for c0 in (0, 512):
                    ps = ppp.tile([P, 512], F32, tag="mm", bufs=2, name="mm")
                    for hd in range(ET):
                        nc.tensor.matmul(ps, wt[:, hd, :],
                                         attnT[:, hd, c0:c0 + 512],
                                         start=(hd == 0), stop=(hd == ET - 1))
                    xc = postg.tile([P, 512], F32, tag="ostg", name="ostg")
                    nc.sync.dma_start(
                        out=xc,
                        in_=d["x_chunk"][et * P:(et + 1) * P, c0:c0 + 512])
                    nc.vector.scalar_tensor_tensor(
                        x2_h[c0 // 512][:, et, :], in0=ps,
                        scalar=bproj_sb[:, et:et + 1], in1=xc,
                        op0=O.add, op1=O.add)
        xn2 = pbig.tile([P, ET, QC], BF16, tag="T4")
        if "proj" in phases:
          with tc.tile_pool(name="pbc2", bufs=4) as pbc, \
             tc.tile_pool(name="pppr2", bufs=1, space="PSUM") as ppp:
            layernorm([(xn2, 0, 1024)], QC, ppp, pbc, None,
                      src_tiles=lambda et, a, n: x2_h[a // 512][:, et,
                                                               a % 512:a % 512 + n])
        # ---- phase 5: FFN (q-chunked halves; Hsb double-buffered in the
        # slots freed by KT and VA so fc1(q1) overlaps fc2(q0)) ----
        if "ffn" in phases:
          Hsb_ab = [pbig.tile([P, HT, 512], BF16, tag="T3", name="HsbA"),
                    pbig.tile([P, HT, 512], BF16, tag="T5", name="HsbB")]
          with tc.tile_pool(name="pwf", bufs=2) as pwf, \
             tc.tile_pool(name="pwf2", bufs=3) as pwf2, \
             tc.tile_pool(name="ppff", bufs=2, space="PSUM") as ppf:
            for qch in (0, 512):
                Hsb = Hsb_ab[qch // 512]
                for hg in range(8):
                    wt = pwf.tile([P, ET, 512], BF16, tag="wfc1", name="wfc1")
                    nc.sync.dma_start(
                        out=wt,
                        in_=d["wfc"][:, hg * 512:(hg + 1) * 512]
                        .rearrange("(t p) c -> p t c", p=P))
                    for h4 in range(4):
                        ht = hg * 4 + h4
                        ps = ppf.tile([P, 512], F32, tag="mmh", name="mmh")
                        for et in range(ET):
                            nc.tensor.matmul(
                                ps, wt[:, et, h4 * P:(h4 + 1) * P],
                                xn2[:, et, qch:qch + 512],
                                start=(et == 0), stop=(et == ET - 1))
                        if gelu_mode == "hw":
                            nc.scalar.activation(Hsb[:, ht, :], ps, A.Gelu,
                                                 bias=bfc_sb[:, ht:ht + 1])
                        else:
                            xb = postg.tile([P, 512], F32, tag="ostg",
                                            name="ostg")
                            nc.scalar.activation(xb, ps, A.Identity,
                                                 bias=bfc_sb[:, ht:ht + 1])
                            t = postg.tile([P, 512], F32, tag="ostg",
                                           name="ostg")
                            nc.vector.tensor_mul(t, xb, xb)
                            nc.vector.tensor_scalar(
                                t, t, 0.035677408136300125, 0.7978845608028654,
                                op0=O.mult, op1=O.add)
                            nc.vector.tensor_mul(t, t, xb)
                            nc.scalar.activation(t, t, A.Tanh)
                            nc.vector.scalar_tensor_tensor(
                                t, in0=t, scalar=1.0, in1=xb,
                                op0=O.add, op1=O.mult)
                            nc.vector.tensor_scalar(
                                Hsb[:, ht, :], t, 0.5, None, op0=O.mult)
                for eg in range(2):
                    psY = [ppf.tile([P, 512], F32, tag="psY", bufs=4,
                                    name=f"psY{i}") for i in range(4)]
                    for ht in range(HT):
                        wt = pwf2.tile([P, 512], BF16, tag="wfc2", name="wfc2")
                        nc.sync.dma_start(
                            out=wt,
                            in_=d["wfc2"][ht * P:(ht + 1) * P,
                                          eg * 512:(eg + 1) * 512])
                        for e4 in range(4):
                            nc.tensor.matmul(
                                psY[e4], wt[:, e4 * P:(e4 + 1) * P],
                                Hsb[:, ht, :],
                                start=(ht == 0), stop=(ht == HT - 1))
                    for e4 in range(4):
                        et = eg * 4 + e4
                        og = postg.tile([P, 512], F32, tag="ostg", name="ostg")
                        nc.vector.scalar_tensor_tensor(
                            og, in0=psY[e4], scalar=bfc2_sb[:, et:et + 1],
                            in1=x2_h[qch // 512][:, et, :],
                            op0=O.add, op1=O.add)
                        nc.sync.dma_start(
                            out=out_ap[et * P:(et + 1) * P, qch:qch + 512],
                            in_=og)

# ---------------------------------------------------------------------------
# host side
# ---------------------------------------------------------------------------

_PROG_CACHE = {}


def get_program(repeat=1, gelu_mode="hw"):
    key = (repeat, gelu_mode)
    if key not in _PROG_CACHE:
        _PROG_CACHE[key] = build_program(repeat, gelu_mode)
    return _PROG_CACHE[key]


def prep_in_maps(x, ln1_g, ln1_b, w_attn, b_attn, w_proj, b_proj,
                 ln2_g, ln2_b, w_fc, b_fc, w_fc2, b_fc2):
    f32 = np.float32
    bf = ml_dtypes.bfloat16
    x = np.asarray(x, f32)
    g1 = np.asarray(ln1_g, f32)[:, None]
    wq = (g1 * np.asarray(w_attn[:, 0:E], f32)) / 8.0
    wk = g1 * np.asarray(w_attn[:, E:2 * E], f32)
    wv = g1 * np.asarray(w_attn[:, 2 * E:3 * E], f32)
    bq = (np.asarray(w_attn[:, 0:E], f32).T @ np.asarray(ln1_b, f32)
          + np.asarray(b_attn[0:E], f32)) / 8.0
    bk = (np.asarray(w_attn[:, E:2 * E], f32).T @ np.asarray(ln1_b, f32)
          + np.asarray(b_attn[E:2 * E], f32))
    bv = (np.asarray(w_attn[:, 2 * E:3 * E], f32).T @ np.asarray(ln1_b, f32)
          + np.asarray(b_attn[2 * E:3 * E], f32))
    g2 = np.asarray(ln2_g, f32)[:, None]
    wfc = g2 * np.asarray(w_fc, f32)
    bfc = np.asarray(w_fc, f32).T @ np.asarray(ln2_b, f32) + np.asarray(b_fc, f32)

    shared = {
        "wq": np.ascontiguousarray(wq.astype(bf)),
        "wk": np.ascontiguousarray(wk.astype(bf)),
        "wv": np.ascontiguousarray(wv.astype(bf)),
        "wproj": np.ascontiguousarray(np.asarray(w_proj, f32).astype(bf)),
        "wfc": np.ascontiguousarray(wfc.astype(bf)),
        "wfc2": np.ascontiguousarray(np.asarray(w_fc2, f32).astype(bf)),
        "bq": np.ascontiguousarray(bq.astype(f32)),
        "bk": np.ascontiguousarray(bk.astype(f32)),
        "bv": np.ascontiguousarray(bv.astype(f32)),
        "bproj": np.ascontiguousarray(np.asarray(b_proj, f32)),
        "bfc": np.ascontiguousarray(bfc.astype(f32)),
        "bfc2": np.ascontiguousarray(np.asarray(b_fc2, f32)),
    }

    # multiplicative 0/1 masks (bf16): [:, :128] applies to even key-tiles,
    # [:, 128:] to odd key-tiles; visible iff query_pos >= key_pos.
    vis = (np.arange(P)[:, None] <= np.arange(P)[None, :]).astype(np.float32)
    masks = []
    for parity in (0, 1):
        m = np.zeros((P, 2 * P), np.float32)
        if parity == 0:
            m[:, 0:P] = vis          # even b: diagonal block
            m[:, P:2 * P] = 0.0     # odd b: first suffix slot fully masked
        else:
            m[:, 0:P] = 1.0         # even b: fully visible
            m[:, P:2 * P] = vis     # odd b: diagonal block
        masks.append(np.ascontiguousarray(m.astype(bf)))

    in_maps = []
    for c in range(N_CORES):
        b, parity = c % BATCH, c // BATCH
        xb = x[b]                                   # [S, E]
        rows = np.concatenate(
            [np.arange(P * (2 * j + parity), P * (2 * j + parity) + P)
             for j in range(8)])
        m = dict(shared)
        xbt = np.ascontiguousarray(xb.T)
        m["x_full_bf"] = np.ascontiguousarray(xbt.astype(bf))
        xct = np.ascontiguousarray(xbt[:, rows])
        m["x_chunk"] = xct
        m["x_chunk_bf"] = np.ascontiguousarray(xct.astype(bf))
        m["mask"] = masks[parity]
        in_maps.append(m)
    return in_maps


def assemble_output(results):
    y = np.empty((BATCH, S, E), np.float32)
    for c in range(N_CORES):
        b, parity = c % BATCH, c // BATCH
        rows = np.concatenate(
            [np.arange(P * (2 * j + parity), P * (2 * j + parity) + P)
             for j in range(8)])
        y[b, rows, :] = results[c]["out"].T
    return y


def kernel(**inputs):
    nc = get_program(1)
    in_maps = prep_in_maps(**inputs)
    res = run_bass_kernel_spmd(nc, in_maps, core_ids=list(range(N_CORES)))
    return assemble_output(res.results)



# revision 12
# speedup vs baseline: 1.0968x; 1.0968x over previous
"""Trainium2 Bass kernel for a pre-LN transformer decode layer.

nn_DecodeLayer: x [4, 2048, 1024] f32, 16 heads, causal attention, 4x MLP.

Sharding: 8 cores = 4 batch x 2 query-shards. Core c handles batch c%4 and
query tiles {2j + c//4 : j in 0..7}. The host PERMUTES each core's sequence
(swapping adjacent 128-tile pairs for parity-1 cores) so that every core's
own query tiles sit at EVEN local positions; the kernel is a single uniform
SPMD program and all per-core differences (x layout, causal masks) are data.

On-chip layout: activations are kept transposed ([e, seq]) so every GEMM
contracts over the partition dim. Attention scores are computed directly in
[key, query] layout (scoresT = K @ Q^T) so softmax'd probabilities feed the
attn@V matmul with no transposes; softmax denominators come free via a
ones-column appended to V. Q is read strided (even local tiles) from the
full-sequence LN output, so layernorm runs once per sublayer. LN gains/
biases and the 1/sqrt(d) scale are folded into weights/biases on the host.
"""

import sys

for _p in ("/opt/trn_rl_repo",):
    if _p not in sys.path:
        sys.path.insert(0, _p)

import numpy as np
import ml_dtypes

import concourse.bass as bass
import concourse.tile as tile
from concourse import bacc, mybir
from concourse.bass_utils import run_bass_kernel_spmd

F32 = mybir.dt.float32
F32R = mybir.dt.float32r
BF16 = mybir.dt.bfloat16

E = 1024          # d_model
S = 2048          # sequence length
BATCH = 4
NH = 16           # heads
HD = 64           # head dim
P = 128
ET = E // P       # 8 e-tiles
QC = 1024         # queries per core
NKT = S // P      # 16 key tiles
FF = 4 * E        # 4096
HT = FF // P      # 32 hidden tiles
N_CORES = 8
EPS = 1e-5


def _segs(q0):
    """Split [q0, 1024) at the PSUM bank boundary (512 f32 cols)."""
    segs = []
    if q0 < 512:
        segs.append((q0, 512 - q0))
    segs.append((max(512, q0), QC - max(512, q0)))
    return segs


def build_program(repeat=1, debug=False):
    nc = bacc.Bacc("TRN2", num_devices=N_CORES)

    d = {}
    def din(name, shape, dtype):
        d[name] = nc.dram_tensor(name, shape, dtype, kind="ExternalInput").ap()

    din("x_full_bf", [E, S], BF16)     # x[b].T, bf16, seq tiles permuted
    din("x_chunk", [E, QC], F32)       # this core's query rows (f32 residual)
    din("wq", [E, E], BF16)            # ln1_g-folded, /8-folded
    din("wk", [E, E], BF16)
    din("wv", [E, E], BF16)
    din("wproj", [E, E], BF16)
    din("wfc", [E, FF], BF16)          # ln2_g-folded
    din("wfc2", [FF, E], BF16)
    din("bq", [E], F32)
    din("bk", [E], F32)
    din("bv", [E], F32)
    din("bproj", [E], F32)
    din("bfc", [FF], F32)
    din("bfc2", [E], F32)
    din("mask", [P, 2 * P], BF16)      # multiplicative 0/1 mask
    out_ap = nc.dram_tensor("out", [E, QC], F32, kind="ExternalOutput").ap()
    dbg = {}
    if debug:
        for nm, shape in (("dbg_xnf", [E, S]), ("dbg_kt", [E, S]),
                          ("dbg_qt", [E, QC]), ("dbg_va", [S, NH * (HD + 1)]),
                          ("dbg_attnT", [E, QC]), ("dbg_x2", [E, QC]),
                          ("dbg_xn2", [E, QC])):
            dbg[nm] = nc.dram_tensor(nm, shape, F32 if nm in ("dbg_x2",)
                                     else BF16, kind="ExternalOutput").ap()

    with tile.TileContext(nc) as tc:
        if repeat == 1:
            _emit(nc, tc, d, out_ap, dbg)
        else:
            with tc.For_i(0, repeat, 1):
                _emit(nc, tc, d, out_ap, dbg)

    nc.compile()
    return nc


def _emit(nc, tc, d, out_ap, dbg=None):
    dbg = dbg or {}
    A = mybir.ActivationFunctionType
    O = mybir.AluOpType
    import contextlib
    ctx = contextlib.ExitStack()
    with ctx:
        # --- long-lived pools ---
        pconst = ctx.enter_context(tc.tile_pool(name="pconst", bufs=1))
        pbig = ctx.enter_context(tc.tile_pool(name="pbig", bufs=1))
        prows = ctx.enter_context(tc.tile_pool(name="prows", bufs=1))
        postg = ctx.enter_context(tc.tile_pool(name="postg", bufs=2))
        pbc = ctx.enter_context(tc.tile_pool(name="pbc", bufs=2))

        # --- constants ---
        ones_mat = pconst.tile([P, P], BF16, tag="ones")
        nc.vector.memset(ones_mat, 1.0)
        eps_t = pconst.tile([P, 1], F32, tag="eps")
        nc.vector.memset(eps_t, EPS)
        mask_sb = pconst.tile([P, 2 * P], BF16, tag="mask")
        nc.sync.dma_start(out=mask_sb, in_=d["mask"])

        def bias_cols(name, n_tiles):
            t = pconst.tile([P, n_tiles], F32, tag=f"b_{name}", name=f"b_{name}")
            nc.sync.dma_start(out=t, in_=d[name].rearrange("(t p) -> p t", p=P))
            return t

        bq_sb = bias_cols("bq", ET)
        bk_sb = bias_cols("bk", ET)
        bproj_sb = bias_cols("bproj", ET)
        bfc2_sb = bias_cols("bfc2", ET)
        bfc_sb = bias_cols("bfc", HT)

        # big persistent tiles (tags sized for reuse across phases)
        xnf_h = [pbig.tile([P, ET, 1024], BF16, tag=f"T1{i}", name=f"xnf{i}")
                 for i in range(2)]                      # -> H[0:8], H[8:16]
        attnT = pbig.tile([P, ET, QC], BF16, tag="T2")
        KT = pbig.tile([P, ET, S], BF16, tag="T3")       # -> H[16:32]
        QT = pbig.tile([P, ET, QC], BF16, tag="T4")      # -> xn2
        VA = pbig.tile([P, NKT, NH * (HD + 1)], BF16, tag="T5")  # -> x2

        # V weights resident (DMA overlaps LN1)
        pwv_cm = tc.tile_pool(name="pwv", bufs=1)
        pwv = pwv_cm.__enter__()
        wvf = pwv.tile([P, ET, E], BF16, tag="wvf")
        nc.sync.dma_start(out=wvf, in_=d["wv"].rearrange("(t p) c -> p t c", p=P))
        bvrow = prows.tile([1, E], F32, tag="rows", name="rows")
        nc.sync.dma_start(out=bvrow, in_=d["bv"].rearrange("(o n) -> o n", o=1))
        bvrow_bf = prows.tile([1, E], BF16, tag="rows_bf", name="rows_bf")
        nc.gpsimd.tensor_copy(bvrow_bf, bvrow)
        bvb = pconst.tile([P, E], BF16, tag="bvb")
        nc.gpsimd.partition_broadcast(bvb, bvrow_bf)

        # ---- phase 1: LN1 over the full (permuted) sequence ----
        def layernorm(dst_list, xsrc, pst, half_w=1024):
            # dst_list: list of (dst_ap_for_et(et) ) writer fns per et
            ps_x = pst.tile([P, half_w], F32, tag="st_x", name="st_x")
            ps_q = pst.tile([P, half_w], F32, tag="st_q", name="st_q")
            for et in range(ET):
                xb = xsrc(et)
                sq = pbc.tile([P, half_w], BF16, tag="sq", name="sq")
                if et % 4 == 3:
                    nc.vector.tensor_mul(sq, xb, xb)
                else:
                    nc.scalar.activation(sq, xb, A.Square)
                for c0 in range(0, half_w, 512):
                    sl = slice(c0, c0 + 512)
                    nc.tensor.matmul(ps_x[:, sl], ones_mat, xb[:, sl],
                                     start=(et == 0), stop=(et == ET - 1))
                    nc.tensor.matmul(ps_q[:, sl], ones_mat, sq[:, sl],
                                     start=(et == 0), stop=(et == ET - 1))
            m_bf = pbc.tile([P, half_w], BF16, tag="mbf", bufs=1, name="mbf")
            nc.scalar.activation(m_bf, ps_x, A.Copy, scale=1.0 / E)
            e2 = pbc.tile([P, half_w], F32, tag="e2", bufs=1, name="e2")
            nc.scalar.activation(e2, ps_q, A.Copy, scale=1.0 / E)
            m2 = pbc.tile([P, half_w], F32, tag="m2", bufs=1, name="m2")
            nc.scalar.activation(m2, m_bf, A.Square)
            nc.vector.tensor_sub(e2, e2, m2)
            nc.scalar.activation(e2, e2, A.Sqrt, bias=eps_t)
            rstd = pbc.tile([P, half_w], F32, tag="m2", bufs=1, name="m2")
            nc.vector.reciprocal_approx_fast(rstd, e2)
            rstd_bf = pbc.tile([P, half_w], BF16, tag="rbf", bufs=1, name="rbf")
            nc.vector.tensor_copy(rstd_bf, rstd)
            for et in range(ET):
                xb = xsrc(et)
                dst = dst_list(et)
                eng = nc.gpsimd if et % 4 == 3 else nc.vector
                eng.tensor_sub(dst, xb, m_bf)
                eng.tensor_mul(dst, dst, rstd_bf)

        # x is DMA'd straight into xnf and normalized in place
        with tc.tile_pool(name="pst1", bufs=2, space="PSUM") as pst:
            for h in range(2):
                nc.sync.dma_start(
                    out=xnf_h[h],
                    in_=d["x_full_bf"][:, h * 1024:(h + 1) * 1024]
                    .rearrange("(t p) c -> p t c", p=P))
            for h in range(2):
                layernorm(lambda et, h=h: xnf_h[h][:, et, :],
                          lambda et, h=h: xnf_h[h][:, et, :], pst)

        if "dbg_xnf" in dbg:
            for h in range(2):
                nc.sync.dma_start(
                    out=dbg["dbg_xnf"][:, h * 1024:(h + 1) * 1024]
                    .rearrange("(t p) c -> p t c", p=P), in_=xnf_h[h])

        # ---- phase 2: V (natural layout + ones cols for denominators) ----
        va_hview = VA.rearrange("p t (h c) -> p t h c", c=HD + 1)
        nc.gpsimd.memset(va_hview[:, :, :, HD:HD + 1], 1.0)
        with tc.tile_pool(name="ppv", bufs=3, space="PSUM") as ppv:
            for t in range(NKT):
                xn_src = xnf_h[t // 8]
                for vh in range(2):
                    hbase = vh * (NH // 2)
                    ps = ppv.tile([P, 512], F32, tag="mm", name="mm")
                    for et in range(ET):
                        nc.tensor.matmul(
                            ps, xn_src[:, et, (t % 8) * P:(t % 8 + 1) * P],
                            wvf[:, et, vh * 512:(vh + 1) * 512],
                            start=(et == 0), stop=(et == ET - 1))
                    va_v = va_hview[:, t, hbase:hbase + 8, :]
                    nc.vector.tensor_add(
                        va_v[:, :, 0:HD],
                        ps.rearrange("p (h c) -> p h c", c=HD),
                        bvb[:, vh * 512:(vh + 1) * 512]
                        .rearrange("p (h c) -> p h c", c=HD))

        pwv_cm.__exit__(None, None, None)
        if "dbg_va" in dbg:
            nc.sync.dma_start(
                out=dbg["dbg_va"].rearrange("(t p) c -> p t c", p=P), in_=VA)

        # proj weights resident; DMA here so it overlaps attention
        pwp = ctx.enter_context(tc.tile_pool(name="pwp", bufs=1))
        wpf = pwp.tile([P, ET, E], BF16, tag="wpf")
        nc.sync.dma_start(out=wpf, in_=d["wproj"].rearrange("(t p) c -> p t c", p=P))

        # ---- phase 3: K/Q + attention, software-pipelined ----
        with tc.tile_pool(name="pwk", bufs=2) as pwk, \
             tc.tile_pool(name="pprobs", bufs=3) as pprobs, \
             tc.tile_pool(name="prb", bufs=1) as prb, \
             tc.tile_pool(name="psc", bufs=2, space="PSUM") as psc, \
             tc.tile_pool(name="ppO", bufs=2, space="PSUM") as ppO:

            def q_moving(half, et):
                # even local tiles of xnf half: cols k*256 + 0..127
                return xnf_h[half][:, et, :] \
                    .rearrange("p (a b) -> p a b", b=256)[:, :, 0:128]

            def emit_kq_dma(kd):
                wtk = pwk.tile([P, ET, P], BF16, tag="wck", name="wck")
                nc.sync.dma_start(
                    out=wtk, in_=d["wk"][:, kd * P:(kd + 1) * P]
                    .rearrange("(t p) c -> p t c", p=P))
                wtq = pwk.tile([P, ET, P], BF16, tag="wcq", name="wcq")
                nc.sync.dma_start(
                    out=wtq, in_=d["wq"][:, kd * P:(kd + 1) * P]
                    .rearrange("(t p) c -> p t c", p=P))
                return wtk, wtq

            def emit_k_chunk(kd, c0, wtk):
                ps = psc.tile([P, QC], F32, tag="sc", name="sc")
                for et in range(ET):
                    nc.tensor.matmul(
                        ps[:, 0:512], wtk[:, et, :],
                        xnf_h[c0 // 1024][:, et, c0 % 1024:c0 % 1024 + 512],
                        start=(et == 0), stop=(et == ET - 1))
                nc.vector.tensor_scalar(
                    KT[:, kd, c0:c0 + 512], ps[:, 0:512],
                    bk_sb[:, kd:kd + 1], None, op0=O.add)

            def emit_q_chunk(kd, c0, wtq):
                ps = psc.tile([P, QC], F32, tag="sc", name="sc")
                for et in range(ET):
                    nc.tensor.matmul(ps[:, 0:512], wtq[:, et, :],
                                     q_moving(c0 // 512, et),
                                     start=(et == 0), stop=(et == ET - 1))
                nc.vector.tensor_scalar(
                    QT[:, kd, c0:c0 + 512], ps[:, 0:512],
                    bq_sb[:, kd:kd + 1], None, op0=O.add)

            def make_kq_thunks(kd):
                wtk, wtq = emit_kq_dma(kd)
                th = []
                for c0 in range(0, S, 512):
                    th.append(lambda kd=kd, c0=c0, w=wtk: emit_k_chunk(kd, c0, w))
                for c0 in (0, 512):
                    th.append(lambda kd=kd, c0=c0, w=wtq: emit_q_chunk(kd, c0, w))
                return th

            def emit_head(h):
                kdt, off = h // 2, (h % 2) * HD
                psO = ppO.tile([HD + 1, QC], F32, tag="psO", name="psO")
                for b in range(NKT):
                    q0 = (b // 2) * P
                    qlen = QC - q0
                    base = (q0 // 512) * 512
                    ps = psc.tile([P, QC], F32, tag="sc", name="sc")
                    for (s0, ln) in _segs(q0):
                        nc.tensor.matmul(
                            ps[:, s0 - base:s0 - base + ln],
                            KT[off:off + HD, kdt, b * P:(b + 1) * P],
                            QT[off:off + HD, kdt, s0:s0 + ln],
                            start=True, stop=True)
                    probs = pprobs.tile([P, QC], BF16, tag="probs",
                                        name="probs")
                    nc.scalar.activation(probs[:, 0:qlen],
                                         ps[:, q0 - base:q0 - base + qlen],
                                         A.Exp)
                    mvar = mask_sb[:, (b % 2) * P:(b % 2 + 1) * P]
                    eng = nc.vector if b % 2 == 0 else nc.gpsimd
                    eng.tensor_mul(probs[:, 0:P], probs[:, 0:P], mvar)
                    for (s0, ln) in _segs(q0):
                        last = 7 if s0 < 512 else 15
                        nc.tensor.matmul(
                            psO[:, s0:s0 + ln],
                            VA[:, b, h * (HD + 1):(h + 1) * (HD + 1)],
                            probs[:, s0 - q0:s0 - q0 + ln],
                            start=(b == 0), stop=(b == last),
                            skip_group_check=True)
                srow = prows.tile([1, QC], F32, tag="rows", name="rows")
                nc.vector.tensor_copy(srow, psO[HD:HD + 1, :])
                srow2 = prows.tile([1, QC], F32, tag="rows2", name="rows2")
                nc.vector.reciprocal_approx_fast(srow2, srow)
                srow_bf = prows.tile([1, QC], BF16, tag="rows_bf",
                                     name="rows_bf")
                nc.vector.tensor_copy(srow_bf, srow2)
                rb = prb.tile([HD, QC], BF16, tag="rb", name="rb")
                nc.gpsimd.partition_broadcast(rb, srow_bf)
                nc.vector.tensor_mul(attnT[off:off + HD, kdt, :],
                                     psO[0:HD, :], rb)

            pending = make_kq_thunks(0)
            for th in pending:
                th()
            pending = []
            for kd in range(ET):
                if kd + 1 < ET:
                    pending = make_kq_thunks(kd + 1)
                for h in (2 * kd, 2 * kd + 1):
                    emit_head(h)
                    n = 3 if h % 2 == 0 else len(pending)
                    for th in pending[:n]:
                        th()
                    pending = pending[n:]

        if "dbg_kt" in dbg:
            nc.sync.dma_start(
                out=dbg["dbg_kt"].rearrange("(t p) c -> p t c", p=P), in_=KT)
        if "dbg_qt" in dbg:
            nc.sync.dma_start(
                out=dbg["dbg_qt"].rearrange("(t p) c -> p t c", p=P), in_=QT)
        if "dbg_attnT" in dbg:
            nc.sync.dma_start(
                out=dbg["dbg_attnT"].rearrange("(t p) c -> p t c", p=P),
                in_=attnT)

        # ---- phase 4: proj + residual -> x2; LN2 -> xn2 (per 512-chunk) ----
        x2 = pbig.tile([P, ET, QC], F32, tag="T5", name="x2")
        xn2 = pbig.tile([P, ET, QC], BF16, tag="T4", name="xn2")
        with tc.tile_pool(name="pppr", bufs=1, space="PSUM") as ppp, \
             tc.tile_pool(name="pst2", bufs=2, space="PSUM") as pst2, \
             tc.tile_pool(name="px2b", bufs=2) as px2b:
            for c0 in (0, 512):
                x2bf = px2b.tile([P, ET, 512], BF16, tag="x2bf", name="x2bf")
                for et in range(ET):
                    ps = ppp.tile([P, 512], F32, tag="mm", bufs=2, name="mm")
                    for hd in range(ET):
                        nc.tensor.matmul(ps, wpf[:, hd, et * P:(et + 1) * P],
                                         attnT[:, hd, c0:c0 + 512],
                                         start=(hd == 0), stop=(hd == ET - 1))
                    xc = postg.tile([P, 512], F32, tag="ostg", name="ostg")
                    nc.sync.dma_start(
                        out=xc,
                        in_=d["x_chunk"][et * P:(et + 1) * P, c0:c0 + 512])
                    nc.vector.scalar_tensor_tensor(
                        x2[:, et, c0:c0 + 512], in0=ps,
                        scalar=bproj_sb[:, et:et + 1], in1=xc,
                        op0=O.add, op1=O.add)
                    nc.scalar.activation(x2bf[:, et, :],
                                         x2[:, et, c0:c0 + 512], A.Copy)
                layernorm(lambda et, c0=c0: xn2[:, et, c0:c0 + 512],
                          lambda et, xb=x2bf: xb[:, et, :], pst2, half_w=512)

        if "dbg_x2" in dbg:
            nc.sync.dma_start(
                out=dbg["dbg_x2"].rearrange("(t p) c -> p t c", p=P), in_=x2)
        if "dbg_xn2" in dbg:
            nc.sync.dma_start(
                out=dbg["dbg_xn2"].rearrange("(t p) c -> p t c", p=P), in_=xn2)

        # ---- phase 5: FFN ----
        def H(ht):
            if ht < 8:
                return xnf_h[0][:, ht, :]
            if ht < 16:
                return xnf_h[1][:, ht - 8, :]
            k = ht - 16
            return KT[:, k // 2, (k % 2) * 1024:(k % 2 + 1) * 1024]

        with tc.tile_pool(name="pwf", bufs=2) as pwf, \
             tc.tile_pool(name="ppff", bufs=2, space="PSUM") as ppf:
            for hg in range(8):
                wt = pwf.tile([P, ET, 512], BF16, tag="wfc1", name="wfc1")
                nc.sync.dma_start(
                    out=wt,
                    in_=d["wfc"][:, hg * 512:(hg + 1) * 512]
                    .rearrange("(t p) c -> p t c", p=P))
                for h4 in range(4):
                    ht = hg * 4 + h4
                    psA = ppf.tile([P, 512], F32, tag="mmA", name="mmA")
                    psB = ppf.tile([P, 512], F32, tag="mmB", name="mmB")
                    for et in range(ET):
                        st = wt[:, et, h4 * P:(h4 + 1) * P]
                        nc.tensor.matmul(psA, st, xn2[:, et, 0:512],
                                         start=(et == 0), stop=(et == ET - 1))
                        nc.tensor.matmul(psB, st, xn2[:, et, 512:1024],
                                         start=(et == 0), stop=(et == ET - 1))
                    nc.scalar.activation(H(ht)[:, 0:512], psA, A.Gelu,
                                         bias=bfc_sb[:, ht:ht + 1])
                    nc.scalar.activation(H(ht)[:, 512:1024], psB, A.Gelu,
                                         bias=bfc_sb[:, ht:ht + 1])

        with tc.tile_pool(name="pwf2", bufs=4) as pwf2, \
             tc.tile_pool(name="ppff2", bufs=1, space="PSUM") as ppf2:
            for eg in range(2):
                psY = [ppf2.tile([P, QC], F32, tag=f"psY{i}", name=f"psY{i}")
                       for i in range(4)]
                for ht in range(HT):
                    wt = pwf2.tile([P, 512], BF16, tag="wfc2", name="wfc2")
                    nc.sync.dma_start(
                        out=wt,
                        in_=d["wfc2"][ht * P:(ht + 1) * P,
                                      eg * 512:(eg + 1) * 512])
                    for e4 in range(4):
                        for qch in (0, 512):
                            nc.tensor.matmul(
                                psY[e4][:, qch:qch + 512],
                                wt[:, e4 * P:(e4 + 1) * P],
                                H(ht)[:, qch:qch + 512],
                                start=(ht == 0), stop=(ht == HT - 1))
                for e4 in range(4):
                    et = eg * 4 + e4
                    for qch in (0, 512):
                        og = postg.tile([P, 512], F32, tag="ostg", name="ostg")
                        nc.vector.scalar_tensor_tensor(
                            og, in0=psY[e4][:, qch:qch + 512],
                            scalar=bfc2_sb[:, et:et + 1],
                            in1=x2[:, et, qch:qch + 512],
                            op0=O.add, op1=O.add)
                        nc.sync.dma_start(
                            out=out_ap[et * P:(et + 1) * P, qch:qch + 512],
                            in_=og)

# ---------------------------------------------------------------------------
# host side
# ---------------------------------------------------------------------------

_PROG_CACHE = {}


def get_program(repeat=1):
    key = repeat
    if key not in _PROG_CACHE:
        _PROG_CACHE[key] = build_program(repeat)
    return _PROG_CACHE[key]


def prep_in_maps(x, ln1_g, ln1_b, w_attn, b_attn, w_proj, b_proj,
                 ln2_g, ln2_b, w_fc, b_fc, w_fc2, b_fc2):
    f32 = np.float32
    bf = ml_dtypes.bfloat16
    x = np.asarray(x, f32)
    g1 = np.asarray(ln1_g, f32)[:, None]
    wq = (g1 * np.asarray(w_attn[:, 0:E], f32)) / 8.0
    wk = g1 * np.asarray(w_attn[:, E:2 * E], f32)
    wv = g1 * np.asarray(w_attn[:, 2 * E:3 * E], f32)
    bq = (np.asarray(w_attn[:, 0:E], f32).T @ np.asarray(ln1_b, f32)
          + np.asarray(b_attn[0:E], f32)) / 8.0
    bk = (np.asarray(w_attn[:, E:2 * E], f32).T @ np.asarray(ln1_b, f32)
          + np.asarray(b_attn[E:2 * E], f32))
    bv = (np.asarray(w_attn[:, 2 * E:3 * E], f32).T @ np.asarray(ln1_b, f32)
          + np.asarray(b_attn[2 * E:3 * E], f32))
    g2 = np.asarray(ln2_g, f32)[:, None]
    wfc = g2 * np.asarray(w_fc, f32)
    bfc = np.asarray(w_fc, f32).T @ np.asarray(ln2_b, f32) + np.asarray(b_fc, f32)

    shared = {
        "wq": np.ascontiguousarray(wq.astype(bf)),
        "wk": np.ascontiguousarray(wk.astype(bf)),
        "wv": np.ascontiguousarray(wv.astype(bf)),
        "wproj": np.ascontiguousarray(np.asarray(w_proj, f32).astype(bf)),
        "wfc": np.ascontiguousarray(wfc.astype(bf)),
        "wfc2": np.ascontiguousarray(np.asarray(w_fc2, f32).astype(bf)),
        "bq": np.ascontiguousarray(bq.astype(f32)),
        "bk": np.ascontiguousarray(bk.astype(f32)),
        "bv": np.ascontiguousarray(bv.astype(f32)),
        "bproj": np.ascontiguousarray(np.asarray(b_proj, f32)),
        "bfc": np.ascontiguousarray(bfc.astype(f32)),
        "bfc2": np.ascontiguousarray(np.asarray(b_fc2, f32)),
    }

    # multiplicative 0/1 masks (bf16): [:, :128] applies to even local key
    # tiles, [:, 128:] to odd ones. With the per-core permutation, even b is
    # always the diagonal block; odd b is fully-masked (parity 0) or fully
    # visible (parity 1).
    vis = (np.arange(P)[:, None] <= np.arange(P)[None, :]).astype(np.float32)
    masks = []
    for parity in (0, 1):
        m = np.zeros((P, 2 * P), np.float32)
        m[:, 0:P] = vis
        m[:, P:2 * P] = 0.0 if parity == 0 else 1.0
        masks.append(np.ascontiguousarray(m.astype(bf)))

    # parity-1 cores see the sequence with adjacent 128-tile pairs swapped,
    # so their own query tiles are at even local positions
    perm1 = np.arange(S).reshape(NKT, P)[
        [t ^ 1 for t in range(NKT)]].reshape(-1)

    in_maps = []
    for c in range(N_CORES):
        b, parity = c % BATCH, c // BATCH
        xbt = np.ascontiguousarray(x[b].T)          # [E, S]
        rows = np.concatenate(
            [np.arange(P * (2 * j + parity), P * (2 * j + parity) + P)
             for j in range(8)])
        m = dict(shared)
        xloc = xbt if parity == 0 else xbt[:, perm1]
        m["x_full_bf"] = np.ascontiguousarray(xloc.astype(bf))
        m["x_chunk"] = np.ascontiguousarray(xbt[:, rows])
        m["mask"] = masks[parity]
        in_maps.append(m)
    return in_maps


def assemble_output(results):
    y = np.empty((BATCH, S, E), np.float32)
    for c in range(N_CORES):
        b, parity = c % BATCH, c // BATCH
        rows = np.concatenate(
            [np.arange(P * (2 * j + parity), P * (2 * j + parity) + P)
             for j in range(8)])
        y[b, rows, :] = results[c]["out"].T
    return y


def kernel(**inputs):
    nc = get_program(1)
    in_maps = prep_in_maps(**inputs)
    res = run_bass_kernel_spmd(nc, in_maps, core_ids=list(range(N_CORES)))
    return assemble_output(res.results)
